# revision 44
# baseline (speedup 1.0000x reference)
"""Causal self-attention (B=2, T=2048, E=1024, H=16) on 8 trn2 NeuronCores.

Sharding: tensor-parallel over heads - core c owns heads {2c, 2c+1}.
Each core computes qkv for its 2 heads, causal attention, and a partial
[B, T, E] output projection over its 128 columns of w_proj; the host
sums the 8 partials (bf16 partials, fp32 accumulate).

v3 design (all five engines balanced, seams closed):
  - attention runs on 512-wide q windows; per k-tile the two heads'
    scores matmuls are a ROW-TILED CONCURRENT PAIR (K=64 contraction,
    h0 on PE rows 0-63, h1 on rows 64-127) writing one [128, 2, 512]
    PSUM tile; exp for BOTH heads is ONE ACTIVATE instruction;
  - the causal mask mul covers both heads in ONE DVE instruction;
  - EMISSION interleave: batch-1 qkv chunks 0-2 are emitted inside
    batch-0's attention windows (chunk 3 inside batch-1 window 0), and
    each batch's output-projection group is emitted the moment its
    window's PV accumulation completes - so the PE never starves at the
    batch seam or in the tail (keeps the HAM clock gate at 8/8);
  - psum->sbuf drains are distributed: batch-0 qkv casts + tail proj
    casts on the (otherwise idle) scalar engine, the rest on DVE;
    gpsimd cannot touch PSUM so it handles SBUF-side copies and DMA;
  - proj results stage into a [128, 4, E] bf16 window tile flushed by
    ONE 1MB DMA (outp is bf16; host sums partials in fp32);
  - x / weight loads are single strided-descriptor DMAs per chunk;
    V^T tiles via ONE xbar transpose per chunk (both heads);
  - softmax 1/l: one staging copy releases the PV psum; reciprocal on
    DVE; the partition broadcast is a stride-0 DMA (falls back to
    gpsimd partition_broadcast if unsupported).

Matmul operands are bf16 (PSUM accumulation fp32); the softmax scale
1/8 is folded into w_q on the host.
"""

import numpy as np
import ml_dtypes
from contextlib import ExitStack

import concourse.bass as bass
import concourse.mybir as mybir
import concourse.tile as tile
from concourse import bacc
from concourse.bass_utils import run_bass_kernel_spmd

B, T, E, H, D = 2, 2048, 1024, 16, 64
NCORES = 8
HPC = H // NCORES          # heads per core = 2
JC = HPC * D               # local out-projection columns per core = 128
W = 512                    # q window (one PSUM bank of fp32)
KT = 128                   # k tile (matmul M limit)
NW = T // W                # windows per batch = 4
NKT = T // KT              # k tiles per batch = 16
NCH = 8                    # E/128 contraction chunks

BF16 = mybir.dt.bfloat16
FP32 = mybir.dt.float32
NPBF = ml_dtypes.bfloat16
EXP = mybir.ActivationFunctionType.Exp
CPY = mybir.ActivationFunctionType.Copy

_NC_CACHE = []


def _build_nc():
    nc = bacc.Bacc(None, target_bir_lowering=False)

    # All DRAM layouts are partition-major so every DMA is 128 x 8KB
    # contiguous descriptors (strided layouts cost ~10x engine time).
    xH = nc.dram_tensor("xH", [B, NW, 128, NCH, W], BF16, kind="ExternalInput")
    wqH = nc.dram_tensor("wqH", [3, 128, NCH, JC], BF16, kind="ExternalInput")
    wpT = nc.dram_tensor("wpT", [JC, E], BF16, kind="ExternalInput")
    outp = nc.dram_tensor("outp", [B, NW, 128, 4, E], BF16, kind="ExternalOutput")

    with tile.TileContext(nc) as tc, ExitStack() as ctx:
        const_pool = ctx.enter_context(tc.tile_pool(name="const", bufs=1))
        w_pool = ctx.enter_context(tc.tile_pool(name="w", bufs=1))
        xt_pool = ctx.enter_context(tc.tile_pool(name="xt", bufs=1))
        qk_pool = ctx.enter_context(tc.tile_pool(name="qk", bufs=1))
        va_pool = ctx.enter_context(tc.tile_pool(name="va", bufs=1))
        probs_pool = ctx.enter_context(tc.tile_pool(name="probs", bufs=8))
        outT_pool = ctx.enter_context(tc.tile_pool(name="outT", bufs=1))
        norm_pool = ctx.enter_context(tc.tile_pool(name="norm", bufs=2))
        st_pool = ctx.enter_context(tc.tile_pool(name="st", bufs=3))
        # PSUM: 8 banks = scp 2x2 + ops 1x2 + mmbuf 2x1
        scp_pool = ctx.enter_context(tc.tile_pool(name="scp", bufs=2, space="PSUM"))
        ops_pool = ctx.enter_context(tc.tile_pool(name="ops", bufs=1, space="PSUM"))
        mm_pool = ctx.enter_context(tc.tile_pool(name="mmbuf", bufs=2, space="PSUM"))

        # --- input loads first: the DMA rings run ~86GB/s per queue, so
        # the early chunks are split in halves across queues and issued
        # before any constant setup.  Ring schedule (need-by in parens):
        #   sync:   x0c0A(13) x0c1A(21) x1c0A(27) transposes...
        #   scalar: x0c0B(13) x0c1B(21) then x0c2/x0c3 JIT
        #   gpsimd: wq(14,16,18) x1c0B(27) wp(75) then x1 JIT halves
        wrm = const_pool.tile([1, 8], FP32, tag="wrm")
        nc.gpsimd.memset(wrm[:], 0.0)
        wqfb = [
            w_pool.tile([128, NCH, JC], BF16, tag=f"wq{fb}", name=f"wq{fb}")
            for fb in range(3)
        ]
        for fb in range(3):
            nc.gpsimd.dma_start(wqfb[fb][:], wqH[fb])
        xtc = [
            [
                xt_pool.tile([128, NCH, W], BF16, tag=f"xt{b}{c}", name=f"xt{b}{c}")
                for c in range(NW)
            ]
            for b in range(B)
        ]
        HALF = NCH // 2

        def load_x_half(b, c, half, eng):
            sl = slice(0, HALF) if half == 0 else slice(HALF, NCH)
            eng.dma_start(xtc[b][c][:, sl, :], xH[b, c, :, sl])

        def load_x_chunk(b, c, eng):
            eng.dma_start(xtc[b][c][:], xH[b, c])

        load_x_half(0, 0, 0, nc.sync)
        load_x_half(0, 0, 1, nc.scalar)
        load_x_half(0, 1, 0, nc.sync)
        load_x_half(0, 1, 1, nc.scalar)
        load_x_half(1, 0, 1, nc.gpsimd)
        load_x_half(1, 0, 0, nc.sync)
        wp_sb = w_pool.tile([JC, E], BF16, tag="wp")
        nc.gpsimd.dma_start(wp_sb[:], wpT[:])

        # warmup: pull the exp ACT_TABLE_LOAD (~2.7us) into the prologue
        # (after the scalar-queue DMA issue so it does not delay it)
        wrm2 = const_pool.tile([1, 8], BF16, tag="wrm2")
        nc.scalar.activation(wrm2[:], wrm[:], EXP)

        # mask2[p, h, j] = 1 iff j >= p (causal band for a diagonal tile),
        # same band replicated for both heads so one DVE mul covers both.
        mask2 = const_pool.tile([128, HPC, KT], BF16)
        nc.gpsimd.memset(mask2[:], 1.0)
        for h in range(HPC):
            nc.gpsimd.affine_select(
                out=mask2[:, h, :],
                in_=mask2[:, h, :],
                compare_op=mybir.AluOpType.is_ge,
                fill=0.0,
                base=0,
                channel_multiplier=-1,
                pattern=[[1, KT]],
            )

        # --- per-batch persistent tiles --------------------------------
        # vaug is per (batch, chunk) so a PV's weight load only depends
        # on the chunk that produced its k-tiles
        QT, KTs, vaug, outTt = [], [], [], []
        for b in range(B):
            QT.append(qk_pool.tile([128, T], BF16, tag=f"QT{b}", name=f"QT{b}"))
            KTs.append(qk_pool.tile([128, T], BF16, tag=f"KT{b}", name=f"KT{b}"))
            vac = []
            for c in range(NW):
                va = va_pool.tile(
                    [128, W // KT, HPC, D + 1], BF16,
                    tag=f"va{b}{c}", name=f"va{b}{c}",
                )
                nc.gpsimd.memset(va[:, :, :, D : D + 1], 1.0)
                vac.append(va)
            vaug.append(vac)
            outTt.append(
                outT_pool.tile([128, T], BF16, tag=f"oT{b}", name=f"oT{b}")
            )

        def qkv_fb(b, c, fb, prefetch):
            """ONE filler unit: project x chunk c's fb block (Q/K/V).
            Batch-0 psum drains go to the scalar engine (idle in the
            prologue); batch-1 drains to DVE."""
            csl = slice(c * W, (c + 1) * W)
            if prefetch:  # JIT prefetch of upcoming x chunks (halves
                # across queues; ~6us ring-time per half at ~86GB/s)
                if b == 0 and c + 2 < NW:  # c0/c1 preloaded; fetch c+2
                    load_x_half(0, c + 2, 0, nc.sync)
                    load_x_half(0, c + 2, 1, nc.scalar)
                elif b == 1:
                    # keep the gpsimd ring clear of bulk transfers: the
                    # normalize chain's small lT/rrow DMAs ride it and
                    # were waiting ~6us behind these halves at seams
                    for cn in ([1, 2] if c == 0 else [c + 2]):
                        if cn < NW:
                            load_x_half(1, cn, 0, nc.sync)
                            load_x_half(1, cn, 1, nc.scalar)
            if fb == 2:
                # V^T directly on the PE: stationary = x token-block,
                # moving = V weights -> psum [128 tok, 2*64 vdim].  No
                # xbar transpose (its DMA-queue mode switch serialized
                # whole queues), no staging hops.
                for tb in range(W // KT):
                    pv_ps = mm_pool.tile(
                        [128, HPC, D], FP32, tag="mm", name=f"pv{b}{c}{tb}"
                    )
                    for ct in range(NCH):
                        nc.tensor.matmul(
                            pv_ps[:],
                            xtc[b][c][:, ct, tb * KT : (tb + 1) * KT],
                            wqfb[2][:, ct, :],
                            start=(ct == 0),
                            stop=(ct == NCH - 1),
                        )
                    nc.vector.tensor_copy(
                        vaug[b][c][:, tb, :, 0:D], pv_ps[:]
                    )
                return
            pp = mm_pool.tile([128, W], FP32, tag="mm", name=f"pp{b}{c}{fb}")
            for ct in range(NCH):
                nc.tensor.matmul(
                    pp[:],
                    wqfb[fb][:, ct, :],
                    xtc[b][c][:, ct, :],
                    start=(ct == 0),
                    stop=(ct == NCH - 1),
                )
            pro = b == 0 and c == 0  # prologue chunk: ACT is idle there
            if fb == 0:
                if pro:
                    nc.scalar.activation(QT[b][:, csl], pp[:], CPY)
                else:
                    nc.vector.tensor_copy(QT[b][:, csl], pp[:])
            else:
                if pro:
                    nc.scalar.activation(KTs[b][:, csl], pp[:], CPY)
                else:
                    nc.vector.tensor_copy(KTs[b][:, csl], pp[:])

        def qkv_units(b, c):
            """The three filler units of one qkv chunk (Q, K, V)."""
            return [
                lambda fb=fb: qkv_fb(b, c, fb, prefetch=(fb == 0))
                for fb in (0, 1, 2)
            ]

        def scores_exp(b, w, kt):
            q0 = w * W
            koff = kt * KT - q0
            lo = max(0, koff)
            scp = scp_pool.tile([128, HPC, W], FP32, tag="scp", name="scp")
            ksl = slice(kt * KT, (kt + 1) * KT)
            for h in range(HPC):
                nc.tensor.matmul(
                    scp[:, h, lo:W],
                    KTs[b][h * D : (h + 1) * D, ksl],
                    QT[b][h * D : (h + 1) * D, q0 + lo : q0 + W],
                    start=True,
                    stop=True,
                )
            pr = probs_pool.tile([128, HPC, W], BF16, tag="pr", name="pr")
            nc.scalar.activation(pr[:, :, lo:W], scp[:, :, lo:W], EXP)
            if koff >= 0:  # diagonal tile: mask the 128-band, both heads
                nc.vector.tensor_mul(
                    pr[:, :, lo : lo + KT],
                    pr[:, :, lo : lo + KT],
                    mask2[:],
                )
            return pr, lo

        ops_live = {}

        def pv(b, w, kt, pr, lo):
            """PV accumulate; returns w when this completes window w."""
            nkt = 4 * w + 4
            if kt == 0:
                ops_live[b] = ops_pool.tile(
                    [D + 1, HPC, W], FP32, tag="ops", name=f"ops{b}{w}"
                )
            opsb = ops_live[b]
            for h in range(HPC):
                nc.tensor.matmul(
                    opsb[:, h, lo:W],
                    vaug[b][kt // 4][:, kt % 4, h, :],
                    pr[:, h, lo:W],
                    start=(kt == 0),
                    stop=(kt == nkt - 1),
                    skip_group_check=True,
                )
            if kt == nkt - 1:
                finish_window(b, w, opsb)
                return w
            return None

        def finish_window(b, w, opsb):
            # Deferred normalize: ONE staging copy releases the psum
            # accumulator; the reciprocal/broadcast/mul chain then runs
            # entirely off the critical path from SBUF.
            sg = norm_pool.tile([D + 1, HPC, W], FP32, tag="sg", name=f"sg{b}{w}")
            nc.scalar.activation(sg[:], opsb[0 : D + 1, :, :], CPY)
            # reciprocal directly on the one-partition l row: no DMA
            # round-trip (sync-queue hops serialized the window tails)
            # reshape the one-partition l row to [128, 8] via DMA so the
            # exact reciprocal costs 0.24us (it is 6.5us on one partition)
            lT = norm_pool.tile([128, HPC * W // 128], FP32, tag="lT")
            nc.gpsimd.dma_start(lT[:], sg[D : D + 1, :, :])
            rT = norm_pool.tile([128, HPC * W // 128], FP32, tag="rT")
            nc.vector.reciprocal(rT[:], lT[:])
            # bf16 reciprocal row: the gpsimd DMA casts in flight, and
            # the partition_broadcast (the longest hop in this chain,
            # which head-blocks the DVE FIFO at window seams) moves half
            # the bytes.  Normalize precision cost ~0.4% << 2% gate.
            rrow = norm_pool.tile([1, HPC, W], BF16, tag="rrow")
            nc.gpsimd.dma_start(rrow[:], rT[:])
            bc = norm_pool.tile([D, HPC, W], BF16, tag="bc")
            nc.gpsimd.partition_broadcast(bc[:], rrow[:])
            qsl = slice(w * W, (w + 1) * W)
            for h in range(HPC):
                nc.vector.tensor_mul(
                    outTt[b][h * D : (h + 1) * D, qsl],
                    sg[0:D, h, :],
                    bc[:, h, :],
                )

        def proj_units(b, g, tail=False):
            """Output projection for t blocks 4g..4g+3 of batch b into a
            bf16 staging tile, flushed by ONE strided DMA: four filler
            units (one per t block), the last one also flushes."""
            st_box = []

            def unit(k):
                if not st_box:
                    st_box.append(
                        st_pool.tile([128, 4, E], BF16, tag="st", name=f"st{b}{g}")
                    )
                st = st_box[0]
                tb = 4 * g + k
                for oc in range(2):
                    pj = mm_pool.tile([128, W], FP32, tag="mm", name="pj")
                    nc.tensor.matmul(
                        pj[:],
                        outTt[b][:, tb * 128 : (tb + 1) * 128],
                        wp_sb[:, oc * W : (oc + 1) * W],
                        start=True,
                        stop=True,
                    )
                    if tail:
                        # alternate engines so the MM/cast ping-pong
                        # overlaps even at the (cold) kernel tail
                        te = nc.scalar if (2 * k + oc) % 2 == 0 else nc.vector
                        te.activation(
                            st[:, k, oc * W : (oc + 1) * W], pj[:], CPY
                        ) if te is nc.scalar else te.tensor_copy(
                            st[:, k, oc * W : (oc + 1) * W], pj[:]
                        )
                    else:
                        nc.vector.tensor_copy(
                            st[:, k, oc * W : (oc + 1) * W], pj[:]
                        )
                if tail:
                    # per-token-block flush: the final DMA (which the
                    # drain barrier waits on) is 256KB instead of 512KB
                    eng = nc.sync if k % 2 == 0 else nc.gpsimd
                    eng.dma_start(outp[b, g, :, k : k + 1], st[:, k : k + 1, :])
                elif k == 3:
                    # split the 1MB flush across two queues (~6us each
                    # ring-time; one queue would hold the st slot ~12us)
                    nc.sync.dma_start(outp[b, g, :, 0:2], st[:, 0:2, :])
                    nc.gpsimd.dma_start(outp[b, g, :, 2:4], st[:, 2:4, :])

            return [lambda k=k: unit(k) for k in range(4)]

        # --- emission: per-batch streams; the PV matmuls trail the
        # scores/exp stream by LEAD k-tiles ACROSS window seams so the
        # in-order PE queue never parks on a blocked PV.  The attention
        # spans are ACT-bound (~1.15us/tile exp vs ~0.64us/tile PE), so
        # the PE slack is soaked up by a GLOBAL FIFO of small filler
        # units (~0.9-1.8us each) emitted one per score tile: qkv fb
        # blocks of upcoming chunks (both batches) and proj t-blocks of
        # completed windows.  Fine-grained interleave keeps the in-order
        # PE queue from parking a big filler block ahead of the next
        # window's scores (which starved ACT and dropped the HAM clock
        # gate to 4/8 in earlier versions).
        LEAD = 5
        from collections import deque

        # Two filler queues balance the two ACT-bound attention spans:
        # batch-0's span takes the qkv chunks that must be ready early
        # (~1.7us PE each); batch-1's span takes all proj groups
        # (~0.85us each) plus batch-1's last chunk.  Each span ends up
        # ~10% PE-overcommitted - PE stays dense (HAM warm) and ACT
        # never waits long for a score tile.
        fq_now = deque()   # drained by batch-0's stream
        fq_later = deque() # drained by batch-1's stream (+ tail)

        def batch_stream(b, on_win):
            pending = []

            def fire(ent):
                wdone = pv(*ent)
                if wdone is not None:
                    on_win(wdone)

            for w in range(NW):
                if b == 0:
                    if w + 1 < NW:  # chunk w+1 of batch 0, used next window
                        fq_now.extend(qkv_units(0, w + 1))
                    if w >= 1:  # batch-1 chunks, one window later (3 with 2)
                        fq_now.extend(qkv_units(1, w - 1))
                        if w == NW - 1:
                            fq_now.extend(qkv_units(1, w))
                for kt in range(4 * w + 4):
                    pending.append((b, w, kt, *scores_exp(b, w, kt)))
                    if len(pending) > LEAD:
                        fire(pending.pop(0))
                    if kt < 2:
                        continue
                    q = fq_now if (b == 0 or fq_now) else fq_later
                    if q:
                        q.popleft()()
            for ent in pending:
                fire(ent)

        # prologue: batch-0 chunk 0 emitted as a block (nothing to
        # overlap with yet); Q/K first so window 0's scores start early.
        for u in qkv_units(0, 0):
            u()
        fq_tail = deque()  # reserved: keeps the PE warm (HAM 8/8) while
        # the final window's normalize chain runs, so the last proj
        # group's matmuls are not cold

        def on_win0(w):
            (fq_tail if w == 3 else fq_later).extend(proj_units(0, w))

        batch_stream(0, on_win0)
        batch_stream(1, lambda w: fq_later.extend(proj_units(1, w, tail=(w == 3))))
        for q in (fq_now, fq_tail, fq_later):  # tail
            while q:
                q.popleft()()

    nc.compile()
    return nc


def _get_nc():
    if not _NC_CACHE:
        _NC_CACHE.append(_build_nc())
    return _NC_CACHE[0]


def make_in_maps(x, w_qkv, w_proj):
    x = np.asarray(x, np.float32)
    w_qkv = np.asarray(w_qkv, np.float32)
    w_proj = np.asarray(w_proj, np.float32)
    # xH[b, c, p, i, t] = x[b, c*W + t, i*128 + p]  (partition-major)
    xH = np.ascontiguousarray(
        x.reshape(B, NW, W, NCH, 128).transpose(0, 1, 4, 3, 2)
    ).astype(NPBF)
    in_maps = []
    for c in range(NCORES):
        h0 = c * HPC
        wq = w_qkv[h0 * D : (h0 + HPC) * D] * 0.125  # fold softmax scale
        wk = w_qkv[E + h0 * D : E + (h0 + HPC) * D]
        wv = w_qkv[2 * E + h0 * D : 2 * E + (h0 + HPC) * D]
        wcat = np.stack([wq, wk, wv], 0)  # [3, JC, E]
        # wqH[fb, p, i, j] = wcat[fb, j, i*128 + p]
        wqH = np.ascontiguousarray(
            wcat.transpose(0, 2, 1).reshape(3, NCH, 128, JC).transpose(0, 2, 1, 3)
        )
        wpTc = np.ascontiguousarray(w_proj[:, c * JC : (c + 1) * JC].T)
        in_maps.append(
            {
                "xH": xH,
                "wqH": wqH.astype(NPBF),
                "wpT": wpTc.astype(NPBF),
            }
        )
    return in_maps


def kernel(x, w_qkv, w_proj, **run_kwargs):
    in_maps = make_in_maps(x, w_qkv, w_proj)
    nc = _get_nc()
    res = run_bass_kernel_spmd(nc, in_maps, core_ids=list(range(NCORES)), **run_kwargs)
    # outp[b, g, p, k, e] = partial_out[b, g*W + k*128 + p, e]
    out = np.zeros((B, NW, 128, 4, E), dtype=np.float32)
    for r in res.results:
        out += np.asarray(r["outp"], dtype=np.float32)
    out = out.transpose(0, 1, 3, 2, 4).reshape(B, T, E)
    if run_kwargs:
        kernel.last_results = res
    return out


# revision 45
# speedup vs baseline: 1.0246x; 1.0246x over previous
"""Causal self-attention (B=2, T=2048, E=1024, H=16) on 8 trn2 NeuronCores.

Sharding: tensor-parallel over heads - core c owns heads {2c, 2c+1}.
Each core computes qkv for its 2 heads, causal attention, and a partial
[B, T, E] output projection over its 128 columns of w_proj; the host
sums the 8 partials (bf16 partials, fp32 accumulate).

v3 design (all five engines balanced, seams closed):
  - attention runs on 512-wide q windows; per k-tile the two heads'
    scores matmuls are a ROW-TILED CONCURRENT PAIR (K=64 contraction,
    h0 on PE rows 0-63, h1 on rows 64-127) writing one [128, 2, 512]
    PSUM tile; exp for BOTH heads is ONE ACTIVATE instruction;
  - the causal mask mul covers both heads in ONE DVE instruction;
  - EMISSION interleave: batch-1 qkv chunks 0-2 are emitted inside
    batch-0's attention windows (chunk 3 inside batch-1 window 0), and
    each batch's output-projection group is emitted the moment its
    window's PV accumulation completes - so the PE never starves at the
    batch seam or in the tail (keeps the HAM clock gate at 8/8);
  - psum->sbuf drains are distributed: batch-0 qkv casts + tail proj
    casts on the (otherwise idle) scalar engine, the rest on DVE;
    gpsimd cannot touch PSUM so it handles SBUF-side copies and DMA;
  - proj results stage into a [128, 4, E] bf16 window tile flushed by
    ONE 1MB DMA (outp is bf16; host sums partials in fp32);
  - x / weight loads are single strided-descriptor DMAs per chunk;
    V^T tiles via ONE xbar transpose per chunk (both heads);
  - softmax 1/l: one staging copy releases the PV psum; reciprocal on
    DVE; the partition broadcast is a stride-0 DMA (falls back to
    gpsimd partition_broadcast if unsupported).

Matmul operands are bf16 (PSUM accumulation fp32); the softmax scale
1/8 is folded into w_q on the host.
"""

import numpy as np
import ml_dtypes
from contextlib import ExitStack

import concourse.bass as bass
import concourse.mybir as mybir
import concourse.tile as tile
from concourse import bacc
from concourse.bass_utils import run_bass_kernel_spmd

B, T, E, H, D = 2, 2048, 1024, 16, 64
NCORES = 8
HPC = H // NCORES          # heads per core = 2
JC = HPC * D               # local out-projection columns per core = 128
W = 512                    # q window (one PSUM bank of fp32)
KT = 128                   # k tile (matmul M limit)
NW = T // W                # windows per batch = 4
NKT = T // KT              # k tiles per batch = 16
NCH = 8                    # E/128 contraction chunks

BF16 = mybir.dt.bfloat16
FP32 = mybir.dt.float32
NPBF = ml_dtypes.bfloat16
EXP = mybir.ActivationFunctionType.Exp
CPY = mybir.ActivationFunctionType.Copy

_NC_CACHE = []


def _build_nc():
    nc = bacc.Bacc(None, target_bir_lowering=False)

    # All DRAM layouts are partition-major so every DMA is 128 x 8KB
    # contiguous descriptors (strided layouts cost ~10x engine time).
    xH = nc.dram_tensor("xH", [B, NW, 128, NCH, W], BF16, kind="ExternalInput")
    wqH = nc.dram_tensor("wqH", [3, 128, NCH, JC], BF16, kind="ExternalInput")
    wpT = nc.dram_tensor("wpT", [JC, E], BF16, kind="ExternalInput")
    outp = nc.dram_tensor("outp", [B, NW, 128, 4, E], BF16, kind="ExternalOutput")

    with tile.TileContext(nc) as tc, ExitStack() as ctx:
        const_pool = ctx.enter_context(tc.tile_pool(name="const", bufs=1))
        w_pool = ctx.enter_context(tc.tile_pool(name="w", bufs=1))
        xt_pool = ctx.enter_context(tc.tile_pool(name="xt", bufs=1))
        qk_pool = ctx.enter_context(tc.tile_pool(name="qk", bufs=1))
        va_pool = ctx.enter_context(tc.tile_pool(name="va", bufs=1))
        probs_pool = ctx.enter_context(tc.tile_pool(name="probs", bufs=8))
        outT_pool = ctx.enter_context(tc.tile_pool(name="outT", bufs=1))
        norm_pool = ctx.enter_context(tc.tile_pool(name="norm", bufs=2))
        st_pool = ctx.enter_context(tc.tile_pool(name="st", bufs=3))
        # PSUM: 8 banks = scp 2x2 + ops 1x2 + mmbuf 2x1
        scp_pool = ctx.enter_context(tc.tile_pool(name="scp", bufs=2, space="PSUM"))
        ops_pool = ctx.enter_context(tc.tile_pool(name="ops", bufs=1, space="PSUM"))
        mm_pool = ctx.enter_context(tc.tile_pool(name="mmbuf", bufs=2, space="PSUM"))

        # --- input loads first: the DMA rings run ~86GB/s per queue, so
        # the early chunks are split in halves across queues and issued
        # before any constant setup.  Ring schedule (need-by in parens):
        #   sync:   x0c0A(13) x0c1A(21) x1c0A(27) transposes...
        #   scalar: x0c0B(13) x0c1B(21) then x0c2/x0c3 JIT
        #   gpsimd: wq(14,16,18) x1c0B(27) wp(75) then x1 JIT halves
        wrm = const_pool.tile([1, 8], FP32, tag="wrm")
        nc.gpsimd.memset(wrm[:], 0.0)
        wqfb = [
            w_pool.tile([128, NCH, JC], BF16, tag=f"wq{fb}", name=f"wq{fb}")
            for fb in range(3)
        ]
        for fb in range(3):
            nc.gpsimd.dma_start(wqfb[fb][:], wqH[fb])
        xtc = [
            [
                xt_pool.tile([128, NCH, W], BF16, tag=f"xt{b}{c}", name=f"xt{b}{c}")
                for c in range(NW)
            ]
            for b in range(B)
        ]
        HALF = NCH // 2

        def load_x_half(b, c, half, eng):
            sl = slice(0, HALF) if half == 0 else slice(HALF, NCH)
            eng.dma_start(xtc[b][c][:, sl, :], xH[b, c, :, sl])

        def load_x_chunk(b, c, eng):
            eng.dma_start(xtc[b][c][:], xH[b, c])

        load_x_half(0, 0, 0, nc.sync)
        load_x_half(0, 0, 1, nc.scalar)
        load_x_half(0, 1, 0, nc.sync)
        load_x_half(0, 1, 1, nc.scalar)
        load_x_half(1, 0, 1, nc.gpsimd)
        load_x_half(1, 0, 0, nc.sync)
        wp_sb = w_pool.tile([JC, E], BF16, tag="wp")
        nc.gpsimd.dma_start(wp_sb[:], wpT[:])

        # warmup: pull the exp ACT_TABLE_LOAD (~2.7us) into the prologue
        # (after the scalar-queue DMA issue so it does not delay it)
        wrm2 = const_pool.tile([1, 8], BF16, tag="wrm2")
        nc.scalar.activation(wrm2[:], wrm[:], EXP)

        # mask2[p, h, j] = 1 iff j >= p (causal band for a diagonal tile),
        # same band replicated for both heads so one DVE mul covers both.
        mask2 = const_pool.tile([128, HPC, KT], BF16)
        nc.gpsimd.memset(mask2[:], 1.0)
        for h in range(HPC):
            nc.gpsimd.affine_select(
                out=mask2[:, h, :],
                in_=mask2[:, h, :],
                compare_op=mybir.AluOpType.is_ge,
                fill=0.0,
                base=0,
                channel_multiplier=-1,
                pattern=[[1, KT]],
            )

        # --- per-batch persistent tiles --------------------------------
        # vaug is per (batch, chunk) so a PV's weight load only depends
        # on the chunk that produced its k-tiles
        QT, KTs, vaug, outTt = [], [], [], []
        for b in range(B):
            QT.append(qk_pool.tile([128, T], BF16, tag=f"QT{b}", name=f"QT{b}"))
            KTs.append(qk_pool.tile([128, T], BF16, tag=f"KT{b}", name=f"KT{b}"))
            vac = []
            for c in range(NW):
                va = va_pool.tile(
                    [128, W // KT, HPC, D + 1], BF16,
                    tag=f"va{b}{c}", name=f"va{b}{c}",
                )
                nc.gpsimd.memset(va[:, :, :, D : D + 1], 1.0)
                vac.append(va)
            vaug.append(vac)
            outTt.append(
                outT_pool.tile([128, T], BF16, tag=f"oT{b}", name=f"oT{b}")
            )

        def qkv_fb(b, c, fb, prefetch):
            """ONE filler unit: project x chunk c's fb block (Q/K/V).
            Batch-0 psum drains go to the scalar engine (idle in the
            prologue); batch-1 drains to DVE."""
            csl = slice(c * W, (c + 1) * W)
            if prefetch:  # JIT prefetch of upcoming x chunks (halves
                # across queues; ~6us ring-time per half at ~86GB/s)
                if b == 0 and c + 2 < NW:  # c0/c1 preloaded; fetch c+2
                    load_x_half(0, c + 2, 0, nc.sync)
                    load_x_half(0, c + 2, 1, nc.scalar)
                elif b == 1:
                    # quarters, not halves: the normalize chain's small
                    # lT/rrow DMAs ride the gpsimd ring and wait behind
                    # whatever bulk piece is in flight - 0.25MB caps
                    # that wait at ~3us instead of ~6us
                    for cn in ([1, 2] if c == 0 else [c + 2]):
                        if cn < NW:
                            for qq in range(4):
                                sl = slice(qq * 2, qq * 2 + 2)
                                eng = nc.sync if qq % 2 == 0 else nc.gpsimd
                                eng.dma_start(
                                    xtc[1][cn][:, sl, :], xH[1, cn, :, sl]
                                )
            if fb == 2:
                # V^T directly on the PE: stationary = x token-block,
                # moving = V weights -> psum [128 tok, 2*64 vdim].  No
                # xbar transpose (its DMA-queue mode switch serialized
                # whole queues), no staging hops.
                for tb in range(W // KT):
                    pv_ps = mm_pool.tile(
                        [128, HPC, D], FP32, tag="mm", name=f"pv{b}{c}{tb}"
                    )
                    for ct in range(NCH):
                        nc.tensor.matmul(
                            pv_ps[:],
                            xtc[b][c][:, ct, tb * KT : (tb + 1) * KT],
                            wqfb[2][:, ct, :],
                            start=(ct == 0),
                            stop=(ct == NCH - 1),
                        )
                    nc.vector.tensor_copy(
                        vaug[b][c][:, tb, :, 0:D], pv_ps[:]
                    )
                return
            pp = mm_pool.tile([128, W], FP32, tag="mm", name=f"pp{b}{c}{fb}")
            for ct in range(NCH):
                nc.tensor.matmul(
                    pp[:],
                    wqfb[fb][:, ct, :],
                    xtc[b][c][:, ct, :],
                    start=(ct == 0),
                    stop=(ct == NCH - 1),
                )
            pro = b == 0 and c == 0  # prologue chunk: ACT is idle there
            if fb == 0:
                if pro:
                    nc.scalar.activation(QT[b][:, csl], pp[:], CPY)
                else:
                    nc.vector.tensor_copy(QT[b][:, csl], pp[:])
            else:
                if pro:
                    nc.scalar.activation(KTs[b][:, csl], pp[:], CPY)
                else:
                    nc.vector.tensor_copy(KTs[b][:, csl], pp[:])

        def qkv_units(b, c):
            """The three filler units of one qkv chunk (Q, K, V)."""
            return [
                lambda fb=fb: qkv_fb(b, c, fb, prefetch=(fb == 0))
                for fb in (0, 1, 2)
            ]

        def scores_exp(b, w, kt):
            q0 = w * W
            koff = kt * KT - q0
            lo = max(0, koff)
            scp = scp_pool.tile([128, HPC, W], FP32, tag="scp", name="scp")
            ksl = slice(kt * KT, (kt + 1) * KT)
            for h in range(HPC):
                nc.tensor.matmul(
                    scp[:, h, lo:W],
                    KTs[b][h * D : (h + 1) * D, ksl],
                    QT[b][h * D : (h + 1) * D, q0 + lo : q0 + W],
                    start=True,
                    stop=True,
                )
            pr = probs_pool.tile([128, HPC, W], BF16, tag="pr", name="pr")
            nc.scalar.activation(pr[:, :, lo:W], scp[:, :, lo:W], EXP)
            if koff >= 0:  # diagonal tile: mask the 128-band, both heads
                nc.vector.tensor_mul(
                    pr[:, :, lo : lo + KT],
                    pr[:, :, lo : lo + KT],
                    mask2[:],
                )
            return pr, lo

        ops_live = {}

        def pv(b, w, kt, pr, lo):
            """PV accumulate; returns w when this completes window w."""
            nkt = 4 * w + 4
            if kt == 0:
                ops_live[b] = ops_pool.tile(
                    [D + 1, HPC, W], FP32, tag="ops", name=f"ops{b}{w}"
                )
            opsb = ops_live[b]
            for h in range(HPC):
                nc.tensor.matmul(
                    opsb[:, h, lo:W],
                    vaug[b][kt // 4][:, kt % 4, h, :],
                    pr[:, h, lo:W],
                    start=(kt == 0),
                    stop=(kt == nkt - 1),
                    skip_group_check=True,
                )
            if kt == nkt - 1:
                finish_window(b, w, opsb)
                return w
            return None

        def finish_window(b, w, opsb):
            # Deferred normalize: ONE staging copy releases the psum
            # accumulator; the reciprocal/broadcast/mul chain then runs
            # entirely off the critical path from SBUF.
            sg = norm_pool.tile([D + 1, HPC, W], FP32, tag="sg", name=f"sg{b}{w}")
            nc.scalar.activation(sg[:], opsb[0 : D + 1, :, :], CPY)
            # reciprocal directly on the one-partition l row: no DMA
            # round-trip (sync-queue hops serialized the window tails)
            # reshape the one-partition l row to [128, 8] via DMA so the
            # exact reciprocal costs 0.24us (it is 6.5us on one partition)
            lT = norm_pool.tile([128, HPC * W // 128], FP32, tag="lT")
            nc.gpsimd.dma_start(lT[:], sg[D : D + 1, :, :])
            rT = norm_pool.tile([128, HPC * W // 128], FP32, tag="rT")
            nc.vector.reciprocal(rT[:], lT[:])
            # bf16 reciprocal row: the gpsimd DMA casts in flight, and
            # the partition_broadcast (the longest hop in this chain,
            # which head-blocks the DVE FIFO at window seams) moves half
            # the bytes.  Normalize precision cost ~0.4% << 2% gate.
            rrow = norm_pool.tile([1, HPC, W], BF16, tag="rrow")
            nc.gpsimd.dma_start(rrow[:], rT[:])
            bc = norm_pool.tile([D, HPC, W], BF16, tag="bc")
            nc.gpsimd.partition_broadcast(bc[:], rrow[:])
            qsl = slice(w * W, (w + 1) * W)
            for h in range(HPC):
                nc.vector.tensor_mul(
                    outTt[b][h * D : (h + 1) * D, qsl],
                    sg[0:D, h, :],
                    bc[:, h, :],
                )

        def proj_units(b, g, tail=False):
            """Output projection for t blocks 4g..4g+3 of batch b into a
            bf16 staging tile, flushed by ONE strided DMA: four filler
            units (one per t block), the last one also flushes."""
            st_box = []

            def unit(k):
                if not st_box:
                    st_box.append(
                        st_pool.tile([128, 4, E], BF16, tag="st", name=f"st{b}{g}")
                    )
                st = st_box[0]
                tb = 4 * g + k
                for oc in range(2):
                    pj = mm_pool.tile([128, W], FP32, tag="mm", name="pj")
                    nc.tensor.matmul(
                        pj[:],
                        outTt[b][:, tb * 128 : (tb + 1) * 128],
                        wp_sb[:, oc * W : (oc + 1) * W],
                        start=True,
                        stop=True,
                    )
                    if tail:
                        # alternate engines so the MM/cast ping-pong
                        # overlaps even at the (cold) kernel tail
                        te = nc.scalar if (2 * k + oc) % 2 == 0 else nc.vector
                        te.activation(
                            st[:, k, oc * W : (oc + 1) * W], pj[:], CPY
                        ) if te is nc.scalar else te.tensor_copy(
                            st[:, k, oc * W : (oc + 1) * W], pj[:]
                        )
                    else:
                        nc.vector.tensor_copy(
                            st[:, k, oc * W : (oc + 1) * W], pj[:]
                        )
                if tail:
                    # per-token-block flush: the final DMA (which the
                    # drain barrier waits on) is 256KB instead of 512KB
                    eng = nc.sync if k % 2 == 0 else nc.gpsimd
                    eng.dma_start(outp[b, g, :, k : k + 1], st[:, k : k + 1, :])
                elif k == 3:
                    # split the 1MB flush across two queues (~6us each
                    # ring-time; one queue would hold the st slot ~12us)
                    nc.sync.dma_start(outp[b, g, :, 0:2], st[:, 0:2, :])
                    nc.gpsimd.dma_start(outp[b, g, :, 2:4], st[:, 2:4, :])

            return [lambda k=k: unit(k) for k in range(4)]

        # --- emission: per-batch streams; the PV matmuls trail the
        # scores/exp stream by LEAD k-tiles ACROSS window seams so the
        # in-order PE queue never parks on a blocked PV.  The attention
        # spans are ACT-bound (~1.15us/tile exp vs ~0.64us/tile PE), so
        # the PE slack is soaked up by a GLOBAL FIFO of small filler
        # units (~0.9-1.8us each) emitted one per score tile: qkv fb
        # blocks of upcoming chunks (both batches) and proj t-blocks of
        # completed windows.  Fine-grained interleave keeps the in-order
        # PE queue from parking a big filler block ahead of the next
        # window's scores (which starved ACT and dropped the HAM clock
        # gate to 4/8 in earlier versions).
        LEAD = 5
        from collections import deque

        # Two filler queues balance the two ACT-bound attention spans:
        # batch-0's span takes the qkv chunks that must be ready early
        # (~1.7us PE each); batch-1's span takes all proj groups
        # (~0.85us each) plus batch-1's last chunk.  Each span ends up
        # ~10% PE-overcommitted - PE stays dense (HAM warm) and ACT
        # never waits long for a score tile.
        fq_now = deque()   # drained by batch-0's stream
        fq_later = deque() # drained by batch-1's stream (+ tail)

        def batch_stream(b, on_win):
            pending = []

            def fire(ent):
                wdone = pv(*ent)
                if wdone is not None:
                    on_win(wdone)

            for w in range(NW):
                if b == 0:
                    if w + 1 < NW:  # chunk w+1 of batch 0, used next window
                        fq_now.extend(qkv_units(0, w + 1))
                    if w >= 1:  # batch-1 chunks, one window later (3 with 2)
                        fq_now.extend(qkv_units(1, w - 1))
                        if w == NW - 1:
                            fq_now.extend(qkv_units(1, w))
                for kt in range(4 * w + 4):
                    pending.append((b, w, kt, *scores_exp(b, w, kt)))
                    if len(pending) > LEAD:
                        fire(pending.pop(0))
                    if kt < 2:
                        continue
                    q = fq_now if (b == 0 or fq_now) else fq_later
                    if q:
                        q.popleft()()
            for ent in pending:
                fire(ent)

        # prologue: batch-0 chunk 0 emitted as a block (nothing to
        # overlap with yet); Q/K first so window 0's scores start early.
        for u in qkv_units(0, 0):
            u()
        fq_tail = deque()  # reserved: keeps the PE warm (HAM 8/8) while
        # the final window's normalize chain runs, so the last proj
        # group's matmuls are not cold

        def on_win0(w):
            (fq_tail if w == 3 else fq_later).extend(proj_units(0, w))

        batch_stream(0, on_win0)
        batch_stream(1, lambda w: fq_later.extend(proj_units(1, w, tail=(w == 3))))
        for q in (fq_now, fq_tail, fq_later):  # tail
            while q:
                q.popleft()()

    nc.compile()
    return nc


def _get_nc():
    if not _NC_CACHE:
        _NC_CACHE.append(_build_nc())
    return _NC_CACHE[0]


def make_in_maps(x, w_qkv, w_proj):
    x = np.asarray(x, np.float32)
    w_qkv = np.asarray(w_qkv, np.float32)
    w_proj = np.asarray(w_proj, np.float32)
    # xH[b, c, p, i, t] = x[b, c*W + t, i*128 + p]  (partition-major)
    xH = np.ascontiguousarray(
        x.reshape(B, NW, W, NCH, 128).transpose(0, 1, 4, 3, 2)
    ).astype(NPBF)
    in_maps = []
    for c in range(NCORES):
        h0 = c * HPC
        wq = w_qkv[h0 * D : (h0 + HPC) * D] * 0.125  # fold softmax scale
        wk = w_qkv[E + h0 * D : E + (h0 + HPC) * D]
        wv = w_qkv[2 * E + h0 * D : 2 * E + (h0 + HPC) * D]
        wcat = np.stack([wq, wk, wv], 0)  # [3, JC, E]
        # wqH[fb, p, i, j] = wcat[fb, j, i*128 + p]
        wqH = np.ascontiguousarray(
            wcat.transpose(0, 2, 1).reshape(3, NCH, 128, JC).transpose(0, 2, 1, 3)
        )
        wpTc = np.ascontiguousarray(w_proj[:, c * JC : (c + 1) * JC].T)
        in_maps.append(
            {
                "xH": xH,
                "wqH": wqH.astype(NPBF),
                "wpT": wpTc.astype(NPBF),
            }
        )
    return in_maps


def kernel(x, w_qkv, w_proj, **run_kwargs):
    in_maps = make_in_maps(x, w_qkv, w_proj)
    nc = _get_nc()
    res = run_bass_kernel_spmd(nc, in_maps, core_ids=list(range(NCORES)), **run_kwargs)
    # outp[b, g, p, k, e] = partial_out[b, g*W + k*128 + p, e]
    out = np.zeros((B, NW, 128, 4, E), dtype=np.float32)
    for r in res.results:
        out += np.asarray(r["outp"], dtype=np.float32)
    out = out.transpose(0, 1, 3, 2, 4).reshape(B, T, E)
    if run_kwargs:
        kernel.last_results = res
    return out
